# revision 29
# baseline (speedup 1.0000x reference)
"""Correlation kernel (FlowNet-style, W-displacement only) for Trainium2.

out[b, j, h, w] = mean_c f1[b,c,h,w] * f2pad[b,c,h,w+j],  j in [0, 81), pad=40.

Sharding: data-parallel over batch B=8 across 8 cores (1 batch elem/core).

Design (all bf16 on the wire; ~34MB DMA/core vs 84MB fp32 baseline):
  - Host casts f1*(1/C) and f2 to bf16; device output is bf16, host upcasts.
  - Per h row: 3 matmuls (C=128 contraction on partitions) -> Gram tiles
    G[w, u] in PSUM (fp32), blocks m=108/108/104. The rhs windows read the
    dense (unpadded) f2 row and are clipped at the row edges; the resulting
    structural-zero triangles of the output come from zero stripes that are
    memset once per staging buffer and never overwritten.
    Two consecutive rows share each PSUM tile so evictions batch 2 rows.
  - Eviction copies (fp32->bf16) place each block's Gram in a 188-el slot
    such that the band j in [0,81) of partition p lies at [p, p+81).
  - Band extraction via DRAM bounce (SBUF diagonal APs are illegal): one
    dump DMA per 4-row pack; diagonal readback with partition stride
    pitch+1. (row, block) indices fold into one uniform-stride AP dim
    (row = 3 slots exactly), so one readback covers 6 row-blocks.
  - 3 PE transposes per row (identity matmul, bf16 1 cyc/col) -> [81, 2W]
    bf16 PSUM, copy -> bf16 staging, chunked contiguous DMA out.
  - Software pipelining: pack k's readback/transpose work is emitted after
    pack k+SKEW's matmuls so the bounce round-trip hides behind PE work.
"""

import numpy as np
from contextlib import ExitStack

B, C, H, W = 8, 128, 96, 320
D = 40
J = 2 * D + 1  # 81
N_CORES = 8

HCHUNK = 16
NCHUNK = H // HCHUNK
ROWPACK = 4  # rows per scratch dump DMA
SLOT = 188  # els per block slot; band of partition p at [p, p+81)
ROWELS = 3 * SLOT  # 564
PACKELS = ROWPACK * ROWELS  # 2256
SCRPAD = 64  # readback for p=127 overruns its row; pad the scratch
# blocks: (w0, m lhsT cols, n clipped rhs cols, rhs col offset in dense f2,
#          slot placement offset)
WBLK = [(0, 108, 148, 0, 40), (108, 108, 188, 68, 0), (216, 104, 144, 176, 0)]
PPC = HCHUNK // ROWPACK  # packs per chunk
GBUFS = 4


def _build(h_total=H):
    import concourse.bass as bass
    import concourse.tile as tile
    from concourse import bacc, mybir
    from concourse.masks import make_identity

    bf = mybir.dt.bfloat16
    f32 = mybir.dt.float32
    nc = bacc.Bacc(
        "TRN2",
        target_bir_lowering=False,
        debug=False,
        enable_asserts=False,
        num_devices=N_CORES,
    )
    f1 = nc.dram_tensor("f1", [C, h_total, W], bf, kind="ExternalInput").ap()
    f2 = nc.dram_tensor("f2", [C, h_total, W], bf, kind="ExternalInput").ap()
    out = nc.dram_tensor("out", [J, h_total, W], bf, kind="ExternalOutput").ap()

    nchunk = h_total // HCHUNK
    npack = nchunk * PPC

    with tile.TileContext(nc) as tc, ExitStack() as ctx:
        const_pool = ctx.enter_context(tc.tile_pool(name="const", bufs=1))
        scr_pool = ctx.enter_context(tc.tile_pool(name="scr", bufs=15, space="DRAM"))
        f1_pool = ctx.enter_context(tc.tile_pool(name="f1p", bufs=2))
        f2_pool = ctx.enter_context(tc.tile_pool(name="f2p", bufs=2))
        g_pool = ctx.enter_context(tc.tile_pool(name="gsb", bufs=GBUFS))
        ral_pool = ctx.enter_context(tc.tile_pool(name="ral", bufs=8))
        ost_pool = ctx.enter_context(tc.tile_pool(name="ost", bufs=3))
        pg0_pool = ctx.enter_context(tc.tile_pool(name="pg0", bufs=2, space="PSUM"))
        pg1_pool = ctx.enter_context(tc.tile_pool(name="pg1", bufs=2, space="PSUM"))
        pg2_pool = ctx.enter_context(tc.tile_pool(name="pg2", bufs=2, space="PSUM"))
        pst_pool = ctx.enter_context(tc.tile_pool(name="pst", bufs=2, space="PSUM"))

        ident = const_pool.tile([128, 128], bf)
        make_identity(nc, ident[:])

        chunk_tiles = {}  # ci -> (f1s, f2s, ost)
        pack_state = {}  # k -> (scr, ci, hstart)

        def front(k):
            ci, hp = divmod(k, PPC)
            if hp == 0:
                h0 = ci * HCHUNK
                f1s = f1_pool.tile([C, HCHUNK * W], bf)
                nc.sync.dma_start(f1s[:], f1[:, h0 : h0 + HCHUNK, :])
                f2s = f2_pool.tile([C, HCHUNK * W], bf)
                nc.sync.dma_start(f2s[:], f2[:, h0 : h0 + HCHUNK, :])
                ost = ost_pool.tile([J, HCHUNK * W], bf)
                chunk_tiles[ci] = (f1s, f2s, ost)
            f1s, f2s, ost = chunk_tiles[ci]
            hstart = hp * ROWPACK  # row within chunk

            gsb = g_pool.tile([C, PACKELS], bf)
            if k < GBUFS:
                # zero stripes for the structural-zero output triangles:
                # block0 head [0,40) (rows w+j < 40) and block2 tail
                # [144,188) (rows w+j >= 360). Copies never overwrite these,
                # so each pool buffer needs them only once.
                gv = gsb[:].rearrange("p (r n) -> p r n", r=ROWPACK)
                nc.vector.memset(gv[0:40, :, 0:40], 0.0)
                nc.vector.memset(gv[64:104, :, 2 * SLOT + 144 : 3 * SLOT], 0.0)
            for rp in range(ROWPACK // 2):
                pgs = [
                    pg0_pool.tile([108, 296], f32, tag="a", name="pga"),
                    pg1_pool.tile([108, 376], f32, tag="b", name="pgb"),
                    pg2_pool.tile([104, 288], f32, tag="c", name="pgc"),
                ]
                for r2 in range(2):
                    h = hstart + rp * 2 + r2
                    base1 = h * W
                    for bi, (w0, m, n, roff, soff) in enumerate(WBLK):
                        coff = r2 * n
                        nc.tensor.matmul(
                            pgs[bi][0:m, coff : coff + n],
                            lhsT=f1s[:, base1 + w0 : base1 + w0 + m],
                            rhs=f2s[:, base1 + roff : base1 + roff + n],
                            start=True,
                            stop=True,
                        )
                # eviction copies (2 rows each) into 188-el slots
                base = rp * 2 * ROWELS
                dst = gsb[:, base : base + 2 * ROWELS].rearrange(
                    "p (r n) -> p r n", r=2
                )
                s0 = pgs[0][:, :].rearrange("p (r n) -> p r n", r=2)
                s1 = pgs[1][:, :].rearrange("p (r n) -> p r n", r=2)
                s2 = pgs[2][:, :].rearrange("p (r n) -> p r n", r=2)
                nc.vector.tensor_copy(dst[0:108, :, 40:188], s0)
                nc.scalar.copy(dst[0:108, :, SLOT : SLOT + 188], s1)
                nc.vector.tensor_copy(dst[0:104, :, 2 * SLOT : 2 * SLOT + 144], s2)

            scr = scr_pool.tile([C, PACKELS + SCRPAD], bf)
            nc.scalar.dma_start(scr[:, 0:PACKELS], gsb[:])
            pack_state[k] = (scr, ci, hstart)

        def back(k):
            scr, ci, hstart = pack_state.pop(k)
            _, _, ost = chunk_tiles[ci]
            ss = scr[:]
            # diagonal readback, 2 rows per DMA: (row, block) fold into one
            # uniform-stride dim because ROWELS == 3*SLOT
            ral = ral_pool.tile([C, ROWPACK * 3 * J], bf)
            for rr in range(ROWPACK // 2):
                rsrc = bass.AP(
                    ss.tensor,
                    ss.offset + rr * 2 * ROWELS,
                    [[PACKELS + SCRPAD + 1, 128], [SLOT, 6], [1, J]],
                )
                nc.sync.dma_start(ral[:, rr * 6 * J : (rr + 1) * 6 * J], rsrc)
            for rp in range(ROWPACK // 2):
                pst = pst_pool.tile([J, 2 * W], bf, tag="t")
                for r2 in range(2):
                    r = rp * 2 + r2
                    for bi, (w0, m, n, roff, soff) in enumerate(WBLK):
                        nc.tensor.transpose(
                            pst[0:J, r2 * W + w0 : r2 * W + w0 + m],
                            ral[0:m, (3 * r + bi) * J : (3 * r + bi) * J + J],
                            ident[0:m, 0:m],
                        )
                b1 = (hstart + rp * 2) * W
                nc.scalar.copy(ost[:, b1 : b1 + 2 * W], pst[:])
            if hstart // ROWPACK == PPC - 1:
                h0 = ci * HCHUNK
                nc.sync.dma_start(out[:, h0 : h0 + HCHUNK, :], ost[:])

        SKEW = 12
        for k in range(npack):
            front(k)
            if k >= SKEW:
                back(k - SKEW)
        for k in range(npack - SKEW, npack):
            back(k)

    nc.finalize()
    return nc


def _run(nc, in_maps, **kwargs):
    from concourse.bass_utils import run_bass_kernel_spmd

    return run_bass_kernel_spmd(nc, in_maps, core_ids=list(range(N_CORES)), **kwargs)


def kernel(f1: np.ndarray, f2: np.ndarray, **run_kwargs) -> np.ndarray:
    import ml_dtypes

    assert f1.shape == (B, C, H, W) and f2.shape == (B, C, H, W)
    bf16 = ml_dtypes.bfloat16
    scale = np.float32(1.0 / C)
    nc = _build()
    in_maps = [
        {
            "f1": np.ascontiguousarray(
                (np.asarray(f1[i], dtype=np.float32) * scale).astype(bf16)
            ),
            "f2": np.ascontiguousarray(np.asarray(f2[i], dtype=np.float32).astype(bf16)),
        }
        for i in range(N_CORES)
    ]
    res = _run(nc, in_maps, **run_kwargs)
    out = np.stack([np.asarray(r["out"], dtype=np.float32) for r in res.results], axis=0)
    if run_kwargs:
        kernel.last_results = res
    return out
